# revision 1
# baseline (speedup 1.0000x reference)
"""Trainium2 Bass kernel for nn_ExperimentalEncoder (GC-LSTM encoder + attention-LSTM decoder).

Self-contained: hardcodes B,S,N,F,H = 8,32,1024,4,128 and shards data-parallel
over batch across 8 NeuronCores (1 batch per core, no collectives).

Algebraic structure (validated against the reference numerics):
  - The reference returns the OLD cell state each encoder step, so cell == 0
    throughout: cnew = ig*cs, fg is dead.
  - Decoder softmax is over a size-1 axis == 1.0, so ctx = hseq.sum(S) is a
    constant: accumulate hsum during the encoder, never materialize hseq.
  - The torch-style flat 3-way split of (N*3H,) maps, in feature-major layout
    g1T (3 tiles of (128, N)), to per-residue-class strided column reads.

Layouts on device (per core, feature-major: H on partitions, N on free dim):
  adjT16 (128, 8*1024) f16  : k-tile k cols [1024k,1024k+1024), adjT16[p,1024k+n]=adj[n,128k+p]
  hid16  (128, 8*128)  f16  : node-major k-tiles (stationary for adj matmul)
  all matmuls in fp16 inputs / fp32 PSUM accumulate; elementwise in fp32.
"""
import numpy as np

import concourse.bacc as bacc
import concourse.tile as tile
from concourse import mybir
from concourse.bass_utils import run_bass_kernel_spmd

B, S, N, F, H = 8, 32, 1024, 4, 128
F16, F32 = mybir.dt.float16, mybir.dt.float32
AFT = mybir.ActivationFunctionType

# ---------------------------------------------------------------------------
# gate extraction index math (see header): for flat-chunk gate g in {ig, og},
# destination n = 3m + off reads g1s[tile r] stored col m + s0.
# g1s[j] stores sigmoid of g1T[j] columns [341:1024) compactly (683 cols).
IG_SEGS = [(0, 2, 1), (1, 0, 0), (2, 1, 0)]      # (tile r, off, s0)
OG_SEGS = [(0, 1, 342), (1, 2, 342), (2, 0, 341)]


def _segments(segs, lo, hi):
    """Segments of dst cols [lo,hi): (tile, dst_start, dst_stop, src_lo, count)."""
    out = []
    for r, off, s0 in segs:
        m_lo = -((lo - off) // -3)          # ceil div
        m_hi = (hi - 1 - off) // 3
        cnt = m_hi - m_lo + 1
        if cnt <= 0:
            continue
        d0 = 3 * m_lo + off
        out.append((r, d0, d0 + 3 * (cnt - 1) + 1, s0 + m_lo, cnt))
    return out


def build_program():
    nc = bacc.Bacc("TRN2", target_bir_lowering=False, debug=False)
    d_adjT = nc.dram_tensor("adjT", [128, 8 * N], F16, kind="ExternalInput")
    d_xb = nc.dram_tensor("xb", [128, 8 * S * F], F16, kind="ExternalInput")
    d_w1h = nc.dram_tensor("w1h", [128, 384], F16, kind="ExternalInput")
    d_w1x4 = nc.dram_tensor("w1x4", [128, 128], F16, kind="ExternalInput")
    d_w2h = nc.dram_tensor("w2h", [128, 128], F16, kind="ExternalInput")
    d_b1t = nc.dram_tensor("b1t", [128, 3], F32, kind="ExternalInput")
    d_wd = nc.dram_tensor("wd", [128, 1024], F16, kind="ExternalInput")
    d_id = nc.dram_tensor("ident", [128, 128], F32, kind="ExternalInput")
    d_out = nc.dram_tensor("out", [N, H], F32, kind="ExternalOutput")

    with tile.TileContext(nc) as tc:
        with tc.tile_pool(name="const", bufs=1) as cpool, \
             tc.tile_pool(name="state", bufs=1) as spool:
            adjT = cpool.tile([128, 8 * N], F16)
            xb = cpool.tile([128, 8 * S * F], F16)
            w1h = cpool.tile([128, 384], F16)
            w1x4 = cpool.tile([128, 128], F16)
            w2h = cpool.tile([128, 128], F16)
            b1t = cpool.tile([128, 3], F32)
            wd = cpool.tile([128, 1024], F16)
            ident = cpool.tile([128, 128], F32)
            for t_, d_ in ((adjT, d_adjT), (xb, d_xb), (w1h, d_w1h),
                           (w1x4, d_w1x4), (w2h, d_w2h),
                           (b1t, d_b1t), (wd, d_wd), (ident, d_id)):
                nc.gpsimd.dma_start(t_[:], d_.ap())

            ident16 = spool.tile([128, 128], F16)
            nc.vector.tensor_copy(ident16[:], ident[:])
            hsum = spool.tile([128, N], F32)
            nc.vector.memset(hsum[:], 0.0)
            axt16 = spool.tile([128, N], F16)   # row t*4+f, col n

            # ---------------- phase A: AXT = (adj @ Xb).T, rows t*4+f -------
            with tc.tile_pool(name="encps", bufs=1, space="PSUM") as eps, \
                 tc.tile_pool(name="encsb", bufs=2) as esb, \
                 tc.tile_pool(name="hidp", bufs=2) as hidp, \
                 tc.tile_pool(name="axsp", bufs=3) as axsp:
                axps = eps.tile([128, N], F32, tag="accs")
                for c in range(2):
                    for k in range(8):
                        nc.tensor.matmul(
                            axps[:, 512 * c:512 * c + 512],
                            xb[:, 128 * k:128 * k + 128],
                            adjT[:, 1024 * k + 512 * c:1024 * k + 512 * c + 512],
                            start=(k == 0), stop=(k == 7))
                nc.vector.tensor_copy(axt16[:], axps[:])

                axs = [None] * S
                axs[0] = axsp.tile([128, N], F16, tag="axs", name="axs0")
                for i in range(4):
                    nc.sync.dma_start(axs[0][32 * i:32 * i + 4, :],
                                      axt16[0:4, :])

                # x-side prefill helpers: K=4 matmuls depend only on axs[t],
                # so they run during the previous step's elementwise tail,
                # keeping the PE warm and off the critical path.
                def warmers(ps, n, lo=512, hi=1024):
                    # discardable matmuls to keep the PE HAM busy-window full
                    # during elementwise tails; the following start=True
                    # matmul clears the bank, so results are never read.
                    for _ in range(n):
                        nc.tensor.matmul(ps[:, lo:hi], w1h[:, 0:128],
                                         adjT[:, 0:hi - lo], start=True,
                                         stop=False, skip_group_check=True)

                def prefill_x(t, only):
                    # packed K=4 matmuls: slots j0@row0, j1@row32, cs@row64
                    # run concurrently on disjoint PE row-groups
                    ps0 = eps.tile([128, N], F32, tag="g1", bufs=3,
                                   name=f"psg{t}_0")
                    ps1 = eps.tile([128, N], F32, tag="g1", bufs=3,
                                   name=f"psg{t}_1")
                    psc = eps.tile([128, N], F32, tag="g1", bufs=3,
                                   name=f"pscs{t}")
                    for lo, hi, clo in ((341, 512, 0), (512, 1024, 512)):
                        nc.tensor.matmul(ps0[:, lo:hi], w1x4[0:4, :],
                                         axs[t][0:4, lo:hi], start=True,
                                         stop=only, tile_position=(0, 0))
                        nc.tensor.matmul(ps1[:, lo:hi], w1x4[32:36, :],
                                         axs[t][32:36, lo:hi], start=True,
                                         stop=only, tile_position=(32, 0))
                        nc.tensor.matmul(psc[:, clo:hi], w1x4[64:68, :],
                                         axs[t][64:68, clo:hi], start=True,
                                         stop=only, tile_position=(64, 0))
                    return [ps0, ps1], psc

                def prefill_j2(t, only):
                    ps = eps.tile([128, N], F32, tag="g1", bufs=3,
                                  name=f"psg{t}_2")
                    for lo, hi in ((341, 512), (512, 1024)):
                        nc.tensor.matmul(ps[:, lo:hi], w1x4[96:100, :],
                                         axs[t][96:100, lo:hi], start=True,
                                         stop=only, tile_position=(96, 0))
                    return ps

                def adj_mm(tt, hid_t):
                    ps_ac = eps.tile([128, N], F32, tag="accs",
                                     name=f"psac{tt}")
                    ach = esb.tile([128, N], F16, tag="ach", name=f"ach{tt}")
                    for k in range(8):
                        for c in range(2):
                            nc.tensor.matmul(
                                ps_ac[:, 512 * c:512 * c + 512],
                                hid_t[:, 128 * k:128 * k + 128],
                                adjT[:, 1024 * k + 512 * c:1024 * k + 512 * c + 512],
                                start=(k == 0), stop=(k == 7))
                    for c in range(2):
                        nc.vector.tensor_copy(
                            ach[:, 512 * c:512 * c + 512],
                            ps_ac[:, 512 * c:512 * c + 512])
                    return ach

                # ---------------- phase B: encoder ------------------------
                hid_cur = None
                ach = None
                ps_gs, ps_cs = prefill_x(0, True)
                for t in range(S):
                    first, last = t == 0, t == S - 1
                    if not last:
                        axs[t + 1] = axsp.tile([128, N], F16, tag="axs",
                                               name=f"axs{t+1}")
                        for i in range(4):
                            nc.sync.dma_start(
                                axs[t + 1][32 * i:32 * i + 4, :],
                                axt16[4 * (t + 1):4 * (t + 1) + 4, :])


                    # j2 tile: x-mm in-step (its psum slot frees after sigma0)
                    ps_gs.append(prefill_j2(t, first))
                    g1s = []
                    cst = esb.tile([128, N], F32, tag="cst")

                    def w1h_mms(j):
                        for lo, hi in ((341, 512), (512, 1024)):
                            nc.tensor.matmul(
                                ps_gs[j][:, lo:hi], w1h[:, 128 * j:128 * j + 128],
                                ach[:, lo:hi], start=False, stop=True)

                    def sigma(j):
                        g = esb.tile([128, 683], F32, tag=f"g1s{j}",
                                     name=f"g1s{t}_{j}")
                        nc.scalar.activation(g[:], ps_gs[j][:, 341:1024],
                                             AFT.Sigmoid, bias=b1t[:, j:j + 1])
                        g1s.append(g)

                    def w2h_mms(c):
                        sl = slice(512 * c, 512 * c + 512)
                        nc.tensor.matmul(ps_cs[:, sl], w2h[:], ach[:, sl],
                                         start=False, stop=True)

                    if not first:
                        w1h_mms(0)
                    sigma(0)
                    if not first:
                        w2h_mms(0)
                    nc.scalar.activation(cst[:, 0:512], ps_cs[:, 0:512], AFT.Tanh)
                    if not first:
                        w1h_mms(1)
                    sigma(1)
                    if not first:
                        w1h_mms(2)
                    sigma(2)
                    if not first:
                        w2h_mms(1)
                    nc.scalar.activation(cst[:, 512:1024], ps_cs[:, 512:1024],
                                         AFT.Tanh)

                    # cnew = ig (.) cs ; tanh ; hnew = og (.) tanh(cnew)
                    cnew = esb.tile([128, N], F32, tag="cnew")
                    tcn = esb.tile([128, N], F32, tag="tcn")
                    hnew = esb.tile([128, N], F16, tag="hnew")
                    for c in range(2):
                        lo, hi = 512 * c, 512 * c + 512
                        for r, d0, d1, s0, cnt in _segments(IG_SEGS, lo, hi):
                            nc.vector.tensor_mul(cnew[:, d0:d1:3],
                                                 g1s[r][:, s0:s0 + cnt],
                                                 cst[:, d0:d1:3])
                        nc.scalar.activation(tcn[:, lo:hi], cnew[:, lo:hi], AFT.Tanh)
                        for r, d0, d1, s0, cnt in _segments(OG_SEGS, lo, hi):
                            nc.vector.tensor_mul(hnew[:, d0:d1:3],
                                                 g1s[r][:, s0:s0 + cnt],
                                                 tcn[:, d0:d1:3])

                    # prefill next step's g1/cs between the transpose groups:
                    # interleaved emission staggers PE work across the
                    # elementwise tail so the HAM never sees an idle window
                    ps_gs = []
                    if not last:
                        hid_nxt = hidp.tile([128, N], F16, tag="hid")
                        wt1 = eps.tile([128, N], F32, tag="accs",
                                       name=f"wt1_{t}")
                        warmers(wt1, 6)
                        ps_tr = eps.tile([128, N], F16, tag="accs",
                                         name=f"ps_tr{t}")
                        ps_tr_get = lambda: ps_tr
                        ps_gs, ps_cs = prefill_x(t + 1, False)
                        for k in range(4):
                            sl = slice(128 * k, 128 * k + 128)
                            nc.tensor.transpose(ps_tr_get()[:, sl], hnew[:, sl],
                                                ident16[:])
                        wt2 = eps.tile([128, N], F32, tag="accs",
                                       name=f"wt2_{t}")
                        warmers(wt2, 6)
                        for k in range(4, 8):
                            sl = slice(128 * k, 128 * k + 128)
                            nc.tensor.transpose(ps_tr_get()[:, sl], hnew[:, sl],
                                                ident16[:])
                        for c in range(2):
                            sl = slice(512 * c, 512 * c + 512)
                            nc.vector.tensor_copy(hid_nxt[:, sl],
                                                  ps_tr_get()[:, sl])
                        hid_cur = hid_nxt
                        ach = adj_mm(t + 1, hid_cur)
                    for c in range(2):
                        sl = slice(512 * c, 512 * c + 512)
                        nc.vector.tensor_add(hsum[:, sl], hsum[:, sl],
                                             hnew[:, sl])

            # ---------------- phase C/D: decoder ---------------------------
            hsum16 = spool.tile([128, N], F16)
            nc.vector.tensor_copy(hsum16[:], hsum[:])
            hxf = spool.tile([128, N], F32)

            with tc.tile_pool(name="decps", bufs=1, space="PSUM") as dps, \
                 tc.tile_pool(name="decsb", bufs=2) as dsb:
                hx16 = None
                cx = None

                def prefill_gates(t, only, warm=0):
                    # psum layout per half h: [ig|fg|og|gg] at 2048h + 512j
                    ps = dps.tile([128, 4096], F32, tag="gd", name=f"psgd{t}")
                    for _ in range(warm):
                        nc.tensor.matmul(ps[:, 0:512], wd[:, 0:128],
                                         hsum16[:, 0:512], start=True,
                                         stop=False, skip_group_check=True)
                    for h in range(2):
                        nsl = slice(512 * h, 512 * h + 512)
                        for j in range(4):
                            osl = slice(2048 * h + 512 * j,
                                        2048 * h + 512 * j + 512)
                            nc.tensor.matmul(
                                ps[:, osl],
                                wd[:, 512 + 128 * j:512 + 128 * j + 128],
                                hsum16[:, nsl], start=True, stop=only)
                    return ps

                ps_g = prefill_gates(0, True)
                for t in range(S):
                    first, last = t == 0, t == S - 1
                    sg = dsb.tile([128, 3072], F32, tag="sg")
                    tg = dsb.tile([128, N], F32, tag="tg")
                    m1 = m2 = None
                    if not first:
                        m1 = dsb.tile([128, N], F32, tag="m1", name=f"m1_{t}")
                        m2 = dsb.tile([128, N], F32, tag="m2", name=f"m2_{t}")
                    cx_n = dsb.tile([128, N], F32, tag="cx")
                    tcx = dsb.tile([128, N], F32, tag="tcx")
                    hx_n = (spool.tile([128, N], F32, name="hxf_out") if last
                            else dsb.tile([128, N], F16, tag="hx", name=f"hx{t}"))
                    for h in range(2):
                        nsl = slice(512 * h, 512 * h + 512)
                        if not first:
                            for j in range(4):
                                osl = slice(2048 * h + 512 * j,
                                            2048 * h + 512 * j + 512)
                                nc.tensor.matmul(ps_g[:, osl],
                                                 wd[:, 128 * j:128 * j + 128],
                                                 hx16[:, nsl], start=False, stop=True)
                        nc.scalar.activation(sg[:, 1536 * h:1536 * h + 1536],
                                             ps_g[:, 2048 * h:2048 * h + 1536],
                                             AFT.Sigmoid)
                        nc.scalar.activation(tg[:, nsl],
                                             ps_g[:, 2048 * h + 1536:2048 * h + 2048],
                                             AFT.Tanh)
                        sig_i = sg[:, 1536 * h:1536 * h + 512]
                        sig_f = sg[:, 1536 * h + 512:1536 * h + 1024]
                        sig_o = sg[:, 1536 * h + 1024:1536 * h + 1536]
                        if first:
                            nc.vector.tensor_mul(cx_n[:, nsl], sig_i, tg[:, nsl])
                        else:
                            nc.vector.tensor_mul(m2[:, nsl], sig_i, tg[:, nsl])
                            nc.vector.tensor_mul(m1[:, nsl], sig_f, cx[:, nsl])
                            nc.vector.tensor_add(cx_n[:, nsl], m1[:, nsl], m2[:, nsl])
                        nc.scalar.activation(tcx[:, nsl], cx_n[:, nsl], AFT.Tanh)
                        nc.vector.tensor_mul(hx_n[:, nsl], sig_o, tcx[:, nsl])
                    hx16, cx = hx_n, cx_n
                    if not last:
                        ps_g = prefill_gates(t + 1, False)
                hxf = hx16  # (128, N) f32, feature-major

            # ---------------- phase E: output transpose --------------------
            with tc.tile_pool(name="outps", bufs=2, space="PSUM") as ops, \
                 tc.tile_pool(name="outsb", bufs=1) as osb:
                out_sb = osb.tile([128, N], F32)
                for k in range(8):
                    pt = ops.tile([128, 128], F32, tag="tr")
                    nc.tensor.transpose(pt[:], hxf[:, 128 * k:128 * k + 128],
                                        ident[:])
                    nc.vector.tensor_copy(out_sb[:, 128 * k:128 * k + 128], pt[:])
                nc.sync.dma_start(
                    d_out.ap().rearrange("(k p) h -> p k h", p=128),
                    out_sb[:].rearrange("p (k h) -> p k h", k=8))
    nc.compile()
    return nc


_CACHE = {}


def _get_program():
    if "nc" not in _CACHE:
        _CACHE["nc"] = build_program()
    return _CACHE["nc"]


def _prep_in_maps(x, adj, W1, b1, W2, b2, W_ih, W_hh, b_ih, b_hh):
    f16, f32 = np.float16, np.float32
    adjT16 = np.ascontiguousarray(
        adj.T.reshape(8, 128, N).transpose(1, 0, 2).reshape(128, 8 * N)).astype(f16)
    w1h = W1[4:].astype(f16)
    w2h = W2[4:].astype(f16)
    w1x4 = np.zeros((128, 128), f16)
    w1x4[0:4] = W1[:4, 0:128].astype(f16)
    w1x4[32:36] = W1[:4, 128:256].astype(f16)
    w1x4[64:68] = W2[:4].astype(f16)
    w1x4[96:100] = W1[:4, 256:384].astype(f16)
    b1t = np.ascontiguousarray(b1.reshape(3, 128).T).astype(f32)
    reord = np.r_[0:128, 128:256, 384:512, 256:384]
    wd = np.concatenate([W_hh[reord].T, W_ih[reord].T], axis=1).astype(f16)
    ident = np.eye(128, dtype=f32)
    common = dict(adjT=adjT16, w1h=w1h, w1x4=w1x4, w2h=w2h, b1t=b1t,
                  wd=wd, ident=ident)
    maps = []
    for b in range(B):
        xbn = x[b].transpose(1, 0, 2).reshape(N, S * F)          # (n, t*4+f)
        xb16 = np.ascontiguousarray(
            xbn.reshape(8, 128, S * F).transpose(1, 0, 2).reshape(128, 8 * S * F)
        ).astype(f16)
        maps.append(dict(common, xb=xb16))
    return maps


def run(inputs, trace=False):
    nc = _get_program()
    maps = _prep_in_maps(**{k: np.asarray(v) for k, v in inputs.items()})
    br = run_bass_kernel_spmd(nc, maps, list(range(B)), trace=trace)
    out = np.stack([br.results[c]["out"] for c in range(B)])      # (B, N, H)
    return out.astype(np.float32), br


def kernel(**inputs) -> np.ndarray:
    out, _ = run(inputs, trace=False)
    return out

